# revision 1
# baseline (speedup 1.0000x reference)
"""Multi-head attention (B=2, S=2048, D=1024, H=16, HD=64) on 8 trn2 cores.

Sharding: core c handles batch b = c // 4 and the 4 heads
[4*(c%4), 4*(c%4)+4)  (tensor-parallel split of the Wq/Wk/Wv column dim,
data-parallel over batch).  Each core computes its heads' full SxS
attention locally; no collectives.

Per-core algorithm (all matmuls in fp32r on the PE):
  1. X^T built in SBUF via PE transposes ([d, s] layout, d on partitions).
  2. Q^T, K^T computed as psum = W[k, dout].T-chain over k  -> [dout, s].
     V computed in natural [s, dout] layout (lhsT = X^T tiles), stored
     with a per-head all-ones 65th column for the fused softmax-sum.
  3. Per head, per 1024-wide m-chunk of queries:
       scoresT[t, m] = K^T_h[:, t-tile].T @ Q^T_h   (K = HD = 64)
       expT = exp(scoresT / 8)        (ACT, reads psum directly)
       av[65, m]  += [V_h | 1].T @ expT   (accumulated over t-tiles;
                                           row 64 = softmax denominator)
     then transpose av back to [m, 65] via PE, divide rows by the
     denominator (DVE reciprocal + per-partition scalar multiply) and DMA
     the [128, 64] result blocks to the output.
"""

import os
from contextlib import ExitStack, nullcontext

import numpy as np

import concourse.bacc as bacc
import concourse.mybir as mybir
import concourse.tile as tile
from concourse.bass_utils import run_bass_kernel_spmd
from concourse.masks import make_identity

B, S, D = 2, 2048, 1024
H, HD = 16, 64
NCORES = 8
HPC = H * B // NCORES          # heads per core = 4
HG = HPC * HD                  # per-core projection width = 256
P = 128
KT = D // P                    # 8 contraction tiles
ST = S // P                    # 16 sequence tiles
MC = 1024                      # m-chunk width for the attention loop
NMC = S // MC
VW = HD + 1                    # V columns per head incl. ones column = 65

F32 = mybir.dt.float32
BF16 = mybir.dt.bfloat16
F32R = mybir.dt.float32r
EXP = mybir.ActivationFunctionType.Exp


def _r(ap):
    return ap.bitcast(F32R)


def build_nc(reps=1):
    nc = bacc.Bacc(
        "TRN2", target_bir_lowering=False, debug=False, num_devices=NCORES
    )
    x = nc.dram_tensor("x", [S, D], F32, kind="ExternalInput")
    wq = nc.dram_tensor("wq", [D, HG], F32, kind="ExternalInput")
    wk = nc.dram_tensor("wk", [D, HG], F32, kind="ExternalInput")
    wv = nc.dram_tensor("wv", [D, HG], F32, kind="ExternalInput")
    out = nc.dram_tensor("out", [S, HG], F32, kind="ExternalOutput")

    with tile.TileContext(nc) as tc, ExitStack() as ctx:
        big = ctx.enter_context(tc.tile_pool(name="big", bufs=1))
        xst = ctx.enter_context(tc.tile_pool(name="xst", bufs=4))
        expp = ctx.enter_context(tc.tile_pool(name="expp", bufs=4))
        osbp = ctx.enter_context(tc.tile_pool(name="osbp", bufs=2))
        outp = ctx.enter_context(tc.tile_pool(name="outp", bufs=4))
        recp = ctx.enter_context(tc.tile_pool(name="recp", bufs=4))
        # PSUM budget (8 banks): pp_ss 3 slots x 2 banks = 6 (shared tag for
        # scores/proj/V/transposes -- 3 slots keeps psum slot-reuse waits two
        # steps back so they never drain the PE pipeline), pp_av 1x[65,1024]=2.
        pp_ss = ctx.enter_context(tc.tile_pool(name="pp_ss", bufs=3, space="PSUM"))
        pp_av = ctx.enter_context(tc.tile_pool(name="pp_av", bufs=1, space="PSUM"))

        rep_ctx = tc.For_i(0, reps, 1) if reps > 1 else nullcontext()
        with rep_ctx:
            ident = big.tile([P, P], F32)
            make_identity(nc, ident[:])
            identr = big.tile([P, P], F32)
            nc.vector.tensor_copy(_r(identr[:]), ident[:])

            # ---- persistent SBUF tensors ----
            XT = big.tile([P, KT * S], F32)        # X^T: col(kt, s) = kt*S + s
            WQs = big.tile([P, KT * HG], F32)      # col(kt, j) = kt*HG + j
            WKs = big.tile([P, KT * HG], F32)
            WVs = big.tile([P, KT * HG], F32)
            QT = big.tile([P, 2 * S], F32)         # col(jtile, m) = jtile*S + m
            KTt = big.tile([P, 2 * S], F32)
            Vn = big.tile([P, ST * HPC * VW], BF16)  # col(st, h, e) = st*HPC*VW + h*VW + e

            # ---- load weights (one DMA per weight matrix) ----
            for Wt, w in ((WQs, wq), (WKs, wk), (WVs, wv)):
                nc.sync.dma_start(
                    _r(Wt[:].rearrange("p (k n) -> p k n", n=HG)),
                    _r(w[:].rearrange("(k p) n -> p k n", p=P)),
                )

            # ones columns of Vn (col 64 of each head block). memset can't write
            # f32r, so memset a small f32 tile and round via tensor_copy.
            ones_ap = Vn[:].rearrange("p (s h e) -> p s h e", h=HPC, e=VW)[:, :, :, HD:VW]
            ones_stage = big.tile([P, ST * HPC], F32)
            nc.vector.memset(ones_stage[:], 1.0)
            nc.vector.tensor_copy(
                ones_ap,
                ones_stage[:].rearrange("p (s h e) -> p s h e", h=HPC, e=1),
            )

            # ---- load X (two s-tiles per DMA) and transpose into XT ----
            XT3 = XT[:].rearrange("p (k s) -> p k s", s=S)
            for sp in range(ST // 2):
                xs = xst.tile([P, 2 * D], F32)
                nc.sync.dma_start(
                    _r(xs[:].rearrange("p (t d) -> p t d", d=D)),
                    _r(x[sp * 2 * P:(sp + 1) * 2 * P, :].rearrange("(t p) d -> p t d", p=P)),
                )
                for tt in range(2):
                    st = sp * 2 + tt
                    for g in range(2):
                        pt = pp_ss.tile([P, 512], F32, tag="ps")
                        for j in range(4):
                            kt = g * 4 + j
                            nc.tensor.transpose(
                                _r(pt[:, j * P:(j + 1) * P]),
                                _r(xs[:, tt * D + kt * P: tt * D + (kt + 1) * P]),
                                _r(identr[:]),
                            )
                        dst = XT3[:, g * 4:(g + 1) * 4, st * P:(st + 1) * P]
                        src = pt[:].rearrange("p (k s) -> p k s", s=P)
                        nc.vector.tensor_copy(_r(dst), src)

            # ---- projections / attention, emitted interleaved so the
            # scheduler can fill PE gaps of the ACT-bound attention phase with
            # the later head-group's projection matmuls ----
            Vn4 = Vn[:].rearrange("p (s h e) -> p s h e", h=HPC, e=VW)

            def emit_proj_qk(Wt, Ot, j):
                for nn in range(4):
                    pt = pp_ss.tile([P, 512], F32, tag="ps", name="pt")
                    for kt in range(KT):
                        nc.tensor.matmul(
                            pt[:],
                            _r(Wt[:, kt * HG + j * P: kt * HG + (j + 1) * P]),
                            _r(XT[:, kt * S + nn * 512: kt * S + (nn + 1) * 512]),
                            start=(kt == 0),
                            stop=(kt == KT - 1),
                        )
                    nc.vector.tensor_copy(
                        _r(Ot[:, j * S + nn * 512: j * S + (nn + 1) * 512]), pt[:]
                    )

            def emit_v():
                for st in range(ST):
                    pt = pp_ss.tile([P, HG], F32, tag="ps", name="pt")
                    for kt in range(KT):
                        nc.tensor.matmul(
                            pt[:],
                            _r(XT[:, kt * S + st * P: kt * S + (st + 1) * P]),
                            _r(WVs[:, kt * HG:(kt + 1) * HG]),
                            start=(kt == 0),
                            stop=(kt == KT - 1),
                        )
                    nc.vector.tensor_copy(
                        Vn4[:, st, :, 0:HD], pt[:].rearrange("p (h e) -> p h e", e=HD)
                    )

            def emit_attn(h):
                j = h // 2
                prow = (h % 2) * 64
                qh = QT[prow:prow + 64, j * S:(j + 1) * S]
                kh = KTt[prow:prow + 64, j * S:(j + 1) * S]
                def mm_scores(t, mc):
                    ps = pp_ss.tile([P, MC], F32, tag="ps", name="ps")
                    for hf in range(MC // 512):
                        nc.tensor.matmul(
                            ps[:, hf * 512:(hf + 1) * 512],
                            _r(kh[:, t * P:(t + 1) * P]),
                            _r(qh[:, mc * MC + hf * 512: mc * MC + (hf + 1) * 512]),
                            start=True,
                            stop=True,
                        )
                    return ps

                for mc in range(NMC):
                    av = pp_av.tile([VW, MC], F32, tag="av", name="av")
                    # software pipeline: emit MM_s(t+1) BEFORE exp(t)/MM_av(t)
                    # so the PE never sits behind the ACT exp in program order.
                    ps = mm_scores(0, mc)
                    for t in range(ST):
                        ps_cur = ps
                        if t + 1 < ST:
                            ps = mm_scores(t + 1, mc)
                        ex = expp.tile([P, MC], BF16, tag="ex", name="ex")
                        nc.scalar.activation(ex[:], ps_cur[:], EXP, scale=1.0 / np.sqrt(HD))
                        for hf in range(MC // 512):
                            nc.tensor.matmul(
                                av[:, hf * 512:(hf + 1) * 512],
                                Vn[:, t * HPC * VW + h * VW: t * HPC * VW + (h + 1) * VW],
                                ex[:, hf * 512:(hf + 1) * 512],
                                start=(t == 0),
                                stop=(t == ST - 1),
                            )
                    # evacuate, transpose back, normalize, store (one DMA per
                    # (head, m-chunk))
                    osb = osbp.tile([VW, MC], F32, tag="osb", name="osb")
                    nc.vector.tensor_copy(osb[:], av[:])
                    ot = outp.tile([P, (MC // P) * HD], F32, tag="ot", name="ot")
                    for ms in range(MC // P):
                        po = pp_ss.tile([P, 512], F32, tag="ps", name="po")
                        pot = po[:, 0:VW]
                        nc.tensor.transpose(
                            pot, osb[0:VW, ms * P:(ms + 1) * P], ident[0:VW, 0:VW]
                        )
                        rec = recp.tile([P, 1], F32, tag="rec", name="rec")
                        nc.vector.reciprocal(rec[:], pot[:, HD:VW])
                        nc.vector.tensor_scalar_mul(
                            ot[:, ms * HD:(ms + 1) * HD], pot[:, 0:HD], rec[:]
                        )
                    row0 = mc * MC
                    nc.sync.dma_start(
                        out[row0:row0 + MC, h * HD:(h + 1) * HD].rearrange(
                            "(t p) e -> p t e", p=P
                        ),
                        ot[:].rearrange("p (t e) -> p t e", e=HD),
                    )

            emit_proj_qk(WQs, QT, 0)
            emit_proj_qk(WKs, KTt, 0)
            emit_v()
            emit_attn(0)
            emit_proj_qk(WQs, QT, 1)
            emit_proj_qk(WKs, KTt, 1)
            emit_attn(1)
            emit_attn(2)
            emit_attn(3)

    nc.compile()
    return nc


_NC = None


def _get_nc():
    global _NC
    if _NC is None:
        _NC = build_nc()
    return _NC


def _shard_inputs(inputs, Wq, Wk, Wv):
    inputs = np.ascontiguousarray(np.asarray(inputs, dtype=np.float32))
    Wq = np.asarray(Wq, dtype=np.float32)
    Wk = np.asarray(Wk, dtype=np.float32)
    Wv = np.asarray(Wv, dtype=np.float32)
    in_maps = []
    for c in range(NCORES):
        b, g = c // (NCORES // B), c % (NCORES // B)
        sl = slice(g * HG, (g + 1) * HG)
        in_maps.append(
            {
                "x": inputs[b],
                "wq": np.ascontiguousarray(Wq[:, sl]),
                "wk": np.ascontiguousarray(Wk[:, sl]),
                "wv": np.ascontiguousarray(Wv[:, sl]),
            }
        )
    return in_maps


def _gather(results):
    out = np.empty((B, S, H * HD), dtype=np.float32)
    for c in range(NCORES):
        b, g = c // (NCORES // B), c % (NCORES // B)
        out[b, :, g * HG:(g + 1) * HG] = results[c]["out"]
    return out


def kernel(inputs, Wq, Wk, Wv):
    nc = _get_nc()
    in_maps = _shard_inputs(inputs, Wq, Wk, Wv)
    res = run_bass_kernel_spmd(nc, in_maps, core_ids=list(range(NCORES)))
    return _gather(res.results)



# revision 11
# speedup vs baseline: 1.3107x; 1.3107x over previous
"""Multi-head attention (B=2, S=2048, D=1024, H=16, HD=64) on 8 trn2 cores.

Sharding: core c handles batch b = c // 4 and the 4 heads
[4*(c%4), 4*(c%4)+4)  (tensor-parallel split of the Wq/Wk/Wv column dim,
data-parallel over batch).  Each core computes its heads' full SxS
attention locally; no collectives.

v2 layout (all matmul operands bf16; fp32 PSUM accumulate):
  Phase 1 (DMA/PE pipelined per 256-row X chunk):
    weights DMA'd first and cast to bf16; X chunks DMA'd, PE-transposed
    into XT (bf16), then per-chunk Q/K projections ([128,256] psum tiles,
    m-cols = the chunk) and V projections (natural [s, hd] layout with a
    65th all-ones column for the fused softmax denominator).
  Phase 2 (attention, head-PAIR loop; ACT-bound steady state):
    for each head pair (rows 0-63 / 64-127 of the QT/KTt j-block), each
    1024-wide m-chunk:
      t-loop over 16 key tiles, software-pipelined so ACT (exp) never
      idles: PE regenerates head h's scores while ACT exps head h'.
      scoresT[t,m] psum <- K^T.T @ Q^T (contraction HD=64)
      ex = exp(scoresT/8) (ACT, psum->SBUF bf16)
      av[65,m] += [V|1].T @ ex (row 64 = softmax denominator)
    tail: evacuate av, PE-transpose back to [m,65], DVE reciprocal +
    per-partition scalar multiply, DMA out.
  PSUM budget: pp_ss 2x[128,1024] = 4 banks (scores/proj/transposes),
  pp_av 2x[65,1024] = 4 banks.
"""

import os
from contextlib import ExitStack, nullcontext

import numpy as np

import concourse.bacc as bacc
import concourse.mybir as mybir
import concourse.tile as tile
from concourse.bass_utils import run_bass_kernel_spmd
from concourse.masks import make_identity

B, S, D = 2, 2048, 1024
H, HD = 16, 64
NCORES = 8
HPC = H * B // NCORES          # heads per core = 4
HG = HPC * HD                  # per-core projection width = 256
P = 128
KT = D // P                    # 8 contraction tiles
ST = S // P                    # 16 sequence tiles
MC = 1024                      # m-chunk width for the attention loop
NMC = S // MC
VW = HD + 1                    # V columns per head incl. ones column = 65
NPAIR = HPC // 2               # head pairs per core = 2

F32 = mybir.dt.float32
BF16 = mybir.dt.bfloat16
F32R = mybir.dt.float32r
EXP = mybir.ActivationFunctionType.Exp


def _r(ap):
    return ap.bitcast(F32R)


def build_nc(reps=1):
    nc = bacc.Bacc(
        "TRN2", target_bir_lowering=False, debug=False, num_devices=NCORES
    )
    x = nc.dram_tensor("x", [S, D], F32, kind="ExternalInput")
    wq = nc.dram_tensor("wq", [D, HG], F32, kind="ExternalInput")
    wk = nc.dram_tensor("wk", [D, HG], F32, kind="ExternalInput")
    wv = nc.dram_tensor("wv", [D, HG], F32, kind="ExternalInput")
    out = nc.dram_tensor("out", [S, HG], F32, kind="ExternalOutput")

    with tile.TileContext(nc) as tc, ExitStack() as ctx:
        big = ctx.enter_context(tc.tile_pool(name="big", bufs=1))
        xst = ctx.enter_context(tc.tile_pool(name="xst", bufs=3))
        wst = ctx.enter_context(tc.tile_pool(name="wst", bufs=2))
        expp = ctx.enter_context(tc.tile_pool(name="expp", bufs=4))
        osbp = ctx.enter_context(tc.tile_pool(name="osbp", bufs=2))
        outp = ctx.enter_context(tc.tile_pool(name="outp", bufs=4))
        recp = ctx.enter_context(tc.tile_pool(name="recp", bufs=8))
        # PSUM (8 banks): pp_ss 2x[128,1024]=4 banks shared by proj psums,
        # X transposes, scores, and output transposes; pp_av 2x[65,1024]=4.
        pp_ss = ctx.enter_context(tc.tile_pool(name="pp_ss", bufs=2, space="PSUM"))
        pp_av = ctx.enter_context(tc.tile_pool(name="pp_av", bufs=2, space="PSUM"))

        rep_ctx = tc.For_i(0, reps, 1) if reps > 1 else nullcontext()
        with rep_ctx:
            ident = big.tile([P, P], F32)
            make_identity(nc, ident[:])
            identr = big.tile([P, P], F32)
            nc.vector.tensor_copy(_r(identr[:]), ident[:])

            # ---- persistent SBUF tensors (all bf16) ----
            XT = big.tile([P, KT * S], BF16)       # X^T: col(kt, s) = kt*S + s
            WQb = big.tile([P, KT * HG], BF16)     # col(kt, j) = kt*HG + j
            WKb = big.tile([P, KT * HG], BF16)
            WVb = big.tile([P, KT * HG], BF16)
            QT = big.tile([P, 2 * S], BF16)        # col(j, m) = j*S + m
            KTt = big.tile([P, 2 * S], BF16)
            Vn = big.tile([P, ST * HPC * VW], BF16)  # col(st,h,e) = st*HPC*VW+h*VW+e

            # ---- weights: DMA (f32) then DVE cast to bf16 (interleaved with
            # the first X chunk DMAs below via _emit_w) ----
            def _emit_w(Wb, w):
                ws = wst.tile([P, KT * HG], F32, tag="ws", name="ws")
                nc.sync.dma_start(
                    _r(ws[:].rearrange("p (k n) -> p k n", n=HG)),
                    _r(w[:].rearrange("(k p) n -> p k n", p=P)),
                )
                nc.vector.tensor_copy(Wb[:], ws[:])

            # ones columns of Vn (col 64 of each head block)
            Vn4 = Vn[:].rearrange("p (s h e) -> p s h e", h=HPC, e=VW)
            ones_stage = big.tile([P, ST * HPC], F32)
            nc.vector.memset(ones_stage[:], 1.0)
            nc.vector.tensor_copy(
                Vn4[:, :, :, HD:VW],
                ones_stage[:].rearrange("p (s h e) -> p s h e", h=HPC, e=1),
            )

            # ---- phase 1: X chunks -> transpose -> per-chunk projections ----
            XT3 = XT[:].rearrange("p (k s) -> p k s", s=S)

            def emit_xdma(sp):
                xs = xst.tile([P, 2 * D], F32, tag="xs", name="xs")
                nc.sync.dma_start(
                    _r(xs[:].rearrange("p (t d) -> p t d", d=D)),
                    _r(x[sp * 2 * P:(sp + 1) * 2 * P, :].rearrange(
                        "(t p) d -> p t d", p=P)),
                )
                return xs

            def emit_chunk(sp, xs):
                # transpose into XT (bf16 via the psum->SBUF copy)
                for tt in range(2):
                    st = sp * 2 + tt
                    for g in range(2):
                        pt = pp_ss.tile([P, 512], F32, tag="ps", name="pt")
                        for j in range(4):
                            kt = g * 4 + j
                            nc.tensor.transpose(
                                _r(pt[:, j * P:(j + 1) * P]),
                                _r(xs[:, tt * D + kt * P: tt * D + (kt + 1) * P]),
                                _r(identr[:]),
                            )
                        dst = XT3[:, g * 4:(g + 1) * 4, st * P:(st + 1) * P]
                        src = pt[:].rearrange("p (k s) -> p k s", s=P)
                        nc.vector.tensor_copy(dst, src)
                m0 = sp * 2 * P        # 256 m-cols of this chunk
                # Q/K projections for this chunk's m-cols, both j-blocks
                for Wb, Ot in ((WQb, QT), (WKb, KTt)):
                    for j in range(2):
                        pq = pp_ss.tile([P, 2 * P], F32, tag="ps", name="pq")
                        for kt in range(KT):
                            nc.tensor.matmul(
                                pq[:],
                                Wb[:, kt * HG + j * P: kt * HG + (j + 1) * P],
                                XT[:, kt * S + m0: kt * S + m0 + 2 * P],
                                start=(kt == 0),
                                stop=(kt == KT - 1),
                            )
                        nc.vector.tensor_copy(
                            Ot[:, j * S + m0: j * S + m0 + 2 * P], pq[:]
                        )
                # V projections for this chunk's 2 s-tiles
                for tt in range(2):
                    st = sp * 2 + tt
                    pv = pp_ss.tile([P, HG], F32, tag="ps", name="pv")
                    for kt in range(KT):
                        nc.tensor.matmul(
                            pv[:],
                            XT[:, kt * S + st * P: kt * S + (st + 1) * P],
                            WVb[:, kt * HG:(kt + 1) * HG],
                            start=(kt == 0),
                            stop=(kt == KT - 1),
                        )
                    nc.vector.tensor_copy(
                        Vn4[:, st, :, 0:HD], pv[:].rearrange("p (h e) -> p h e", e=HD)
                    )

            # DMA order: x0, x1, weights, x2..x7 — first transposes start
            # ~2.5us in, weights arrive just before chunk-0 projections.
            xs_pend = [emit_xdma(0), emit_xdma(1)]
            _emit_w(WQb, wq)
            _emit_w(WKb, wk)
            _emit_w(WVb, wv)
            for sp in range(ST // 2):
                if sp + 2 < ST // 2:
                    xs_pend.append(emit_xdma(sp + 2))
                emit_chunk(sp, xs_pend[sp])

            # ---- phase 2: attention over head pairs ----
            def emit_tail_rest(osbs, pr, mc):
                """Transpose back, normalize, store.  Deferred to just after
                the NEXT block's prologue so the tail's PE transposes (which
                wait on the DVE osb copies) never delay the scores the ACT
                exp chain feeds on.  tr tiles share the "av" tag slots."""
                for half in range(2):
                    h = 2 * pr + half
                    osb = osbs[half]
                    tr = pp_av.tile([P, MC], F32, tag="av", name="tr")
                    ot = outp.tile([P, (MC // P) * HD], F32, tag="ot", name="ot")
                    # all transposes back-to-back first: dep tracking is
                    # coarser than the 65-col packing, so interleaving them
                    # with their DVE readers serializes PE<->DVE per block.
                    for ms in range(MC // P):
                        off = (ms % 4) * VW + (ms // 4) * 512
                        nc.tensor.transpose(
                            tr[:, off:off + VW], osb[0:VW, ms * P:(ms + 1) * P],
                            ident[0:VW, 0:VW],
                        )
                    for ms in range(MC // P):
                        off = (ms % 4) * VW + (ms // 4) * 512
                        pot = tr[:, off:off + VW]
                        rec = recp.tile([P, 1], F32, tag="rec", name="rec")
                        nc.vector.reciprocal(rec[:], pot[:, HD:VW])
                        nc.vector.tensor_scalar_mul(
                            ot[:, ms * HD:(ms + 1) * HD], pot[:, 0:HD], rec[:]
                        )
                    row0 = mc * MC
                    nc.sync.dma_start(
                        out[row0:row0 + MC, h * HD:(h + 1) * HD].rearrange(
                            "(t p) e -> p t e", p=P
                        ),
                        ot[:].rearrange("p (t e) -> p t e", e=HD),
                    )

            def emit_attn_pair(pr, mc, pending_tail):
                j = pr
                qk = []
                for half in range(2):
                    prow = half * 64
                    qk.append((
                        QT[prow:prow + 64, j * S:(j + 1) * S],
                        KTt[prow:prow + 64, j * S:(j + 1) * S],
                    ))

                def mm_scores(half, t):
                    qh, kh = qk[half]
                    ps = pp_ss.tile([P, MC], F32, tag="ps", name="ps")
                    for hf in range(MC // 512):
                        nc.tensor.matmul(
                            ps[:, hf * 512:(hf + 1) * 512],
                            kh[:, t * P:(t + 1) * P],
                            qh[:, mc * MC + hf * 512: mc * MC + (hf + 1) * 512],
                            start=True,
                            stop=True,
                        )
                    return ps

                ps_cur = [mm_scores(0, 0), mm_scores(1, 0)]
                if pending_tail is not None:
                    emit_tail_rest(*pending_tail)
                avs = [
                    pp_av.tile([VW, MC], F32, tag="av", name="av")
                    for _ in range(2)
                ]
                osbs = [None, None]
                for t in range(ST):
                    for half in range(2):
                        h = 2 * pr + half
                        ps_t = ps_cur[half]
                        if t + 1 < ST:
                            ps_cur[half] = mm_scores(half, t + 1)
                        ex = expp.tile([P, MC], BF16, tag="ex", name="ex")
                        nc.scalar.activation(
                            ex[:], ps_t[:], EXP, scale=1.0 / np.sqrt(HD)
                        )
                        for hf in range(MC // 512):
                            nc.tensor.matmul(
                                avs[half][:, hf * 512:(hf + 1) * 512],
                                Vn[:, t * HPC * VW + h * VW:
                                   t * HPC * VW + (h + 1) * VW],
                                ex[:, hf * 512:(hf + 1) * 512],
                                start=(t == 0),
                                stop=(t == ST - 1),
                            )
                        if t == ST - 1:
                            # evacuate this half's av as soon as it completes
                            osb = osbp.tile([VW, MC], F32, tag="osb", name="osb")
                            nc.vector.tensor_copy(osb[:], avs[half][:])
                            osbs[half] = osb
                return (osbs, pr, mc)

            pending = None
            for pr in range(NPAIR):
                for mc in range(NMC):
                    pending = emit_attn_pair(pr, mc, pending)
            emit_tail_rest(*pending)

    nc.compile()
    return nc


_NC = None


def _get_nc():
    global _NC
    if _NC is None:
        _NC = build_nc()
    return _NC


def _shard_inputs(inputs, Wq, Wk, Wv):
    inputs = np.ascontiguousarray(np.asarray(inputs, dtype=np.float32))
    Wq = np.asarray(Wq, dtype=np.float32)
    Wk = np.asarray(Wk, dtype=np.float32)
    Wv = np.asarray(Wv, dtype=np.float32)
    in_maps = []
    for c in range(NCORES):
        b, g = c // (NCORES // B), c % (NCORES // B)
        sl = slice(g * HG, (g + 1) * HG)
        in_maps.append(
            {
                "x": inputs[b],
                "wq": np.ascontiguousarray(Wq[:, sl]),
                "wk": np.ascontiguousarray(Wk[:, sl]),
                "wv": np.ascontiguousarray(Wv[:, sl]),
            }
        )
    return in_maps


def _gather(results):
    out = np.empty((B, S, H * HD), dtype=np.float32)
    for c in range(NCORES):
        b, g = c // (NCORES // B), c % (NCORES // B)
        out[b, :, g * HG:(g + 1) * HG] = results[c]["out"]
    return out


def kernel(inputs, Wq, Wk, Wv):
    nc = _get_nc()
    in_maps = _shard_inputs(inputs, Wq, Wk, Wv)
    res = run_bass_kernel_spmd(nc, in_maps, core_ids=list(range(NCORES)))
    return _gather(res.results)


# revision 20
# speedup vs baseline: 1.4396x; 1.0984x over previous
"""Multi-head attention (B=2, S=2048, D=1024, H=16, HD=64) on 8 trn2 cores.

Sharding: core c handles batch b = c // 4 and the 4 heads
[4*(c%4), 4*(c%4)+4)  (tensor-parallel split of the Wq/Wk/Wv column dim,
data-parallel over batch).  Each core computes its heads' full SxS
attention locally; no collectives.

v2 layout (all matmul operands bf16; fp32 PSUM accumulate):
  Phase 1 (DMA/PE pipelined per 256-row X chunk):
    weights DMA'd first and cast to bf16; X chunks DMA'd, PE-transposed
    into XT (bf16), then per-chunk Q/K projections ([128,256] psum tiles,
    m-cols = the chunk) and V projections (natural [s, hd] layout with a
    65th all-ones column for the fused softmax denominator).
  Phase 2 (attention, head-PAIR loop; ACT-bound steady state):
    for each head pair (rows 0-63 / 64-127 of the QT/KTt j-block), each
    1024-wide m-chunk:
      t-loop over 16 key tiles, software-pipelined so ACT (exp) never
      idles: PE regenerates head h's scores while ACT exps head h'.
      scoresT[t,m] psum <- K^T.T @ Q^T (contraction HD=64)
      ex = exp(scoresT/8) (ACT, psum->SBUF bf16)
      av[65,m] += [V|1].T @ ex (row 64 = softmax denominator)
    tail: evacuate av, PE-transpose back to [m,65], DVE reciprocal +
    per-partition scalar multiply, DMA out.
  PSUM budget: pp_ss 2x[128,1024] = 4 banks (scores/proj/transposes),
  pp_av 2x[65,1024] = 4 banks.
"""

import os
from contextlib import ExitStack, nullcontext

import numpy as np

import concourse.bacc as bacc
import concourse.mybir as mybir
import concourse.tile as tile
from concourse.bass_utils import run_bass_kernel_spmd
from concourse.masks import make_identity

B, S, D = 2, 2048, 1024
H, HD = 16, 64
NCORES = 8
HPC = H * B // NCORES          # heads per core = 4
HG = HPC * HD                  # per-core projection width = 256
P = 128
KT = D // P                    # 8 contraction tiles
ST = S // P                    # 16 sequence tiles
MC = 1024                      # m-chunk width for the attention loop
NMC = S // MC
VW = HD + 1                    # V columns per head incl. ones column = 65
NPAIR = HPC // 2               # head pairs per core = 2

F32 = mybir.dt.float32
BF16 = mybir.dt.bfloat16
F32R = mybir.dt.float32r
EXP = mybir.ActivationFunctionType.Exp


def _r(ap):
    return ap.bitcast(F32R)


def build_nc(reps=1):
    nc = bacc.Bacc(
        "TRN2", target_bir_lowering=False, debug=False, num_devices=NCORES
    )
    x = nc.dram_tensor("x", [S, D], F32, kind="ExternalInput")
    wq = nc.dram_tensor("wq", [D, HG], F32, kind="ExternalInput")
    wk = nc.dram_tensor("wk", [D, HG], F32, kind="ExternalInput")
    wv = nc.dram_tensor("wv", [D, HG], F32, kind="ExternalInput")
    out = nc.dram_tensor("out", [S, HG], F32, kind="ExternalOutput")

    with tile.TileContext(nc) as tc, ExitStack() as ctx:
        big = ctx.enter_context(tc.tile_pool(name="big", bufs=1))
        xst = ctx.enter_context(tc.tile_pool(name="xst", bufs=3))
        wst = ctx.enter_context(tc.tile_pool(name="wst", bufs=2))
        expp = ctx.enter_context(tc.tile_pool(name="expp", bufs=4))
        osbp = ctx.enter_context(tc.tile_pool(name="osbp", bufs=2))
        outp = ctx.enter_context(tc.tile_pool(name="outp", bufs=4))
        recp = ctx.enter_context(tc.tile_pool(name="recp", bufs=8))
        # PSUM (8 banks): pp_ss 2x[128,1024]=4 banks shared by proj psums,
        # X transposes, scores, and output transposes; pp_av 2x[65,1024]=4.
        pp_ss = ctx.enter_context(tc.tile_pool(name="pp_ss", bufs=2, space="PSUM"))
        pp_av = ctx.enter_context(tc.tile_pool(name="pp_av", bufs=2, space="PSUM"))

        rep_ctx = tc.For_i(0, reps, 1) if reps > 1 else nullcontext()
        with rep_ctx:
            ident = big.tile([P, P], F32)
            make_identity(nc, ident[:])
            identr = big.tile([P, P], F32)
            nc.vector.tensor_copy(_r(identr[:]), ident[:])

            # ---- persistent SBUF tensors (all bf16) ----
            XT = big.tile([P, KT * S], BF16)       # X^T: col(kt, s) = kt*S + s
            WQb = big.tile([P, KT * HG], BF16)     # col(kt, j) = kt*HG + j
            WKb = big.tile([P, KT * HG], BF16)
            WVb = big.tile([P, KT * HG], BF16)
            QT = big.tile([P, 2 * S], BF16)        # col(j, m) = j*S + m
            KTt = big.tile([P, 2 * S], BF16)
            Vn = big.tile([P, ST * HPC * VW], BF16)  # col(st,h,e) = st*HPC*VW+h*VW+e

            # ---- weights: DMA (f32) then DVE cast to bf16.  DMA and cast are
            # split so the wk/wv casts can be deferred off the critical DVE
            # queue that feeds the first exp. ----
            def _emit_wdma(w):
                ws = wst.tile([P, KT * HG], F32, tag="ws", name="ws", bufs=3)
                nc.sync.dma_start(
                    _r(ws[:].rearrange("p (k n) -> p k n", n=HG)),
                    _r(w[:].rearrange("(k p) n -> p k n", p=P)),
                )
                return ws

            def _emit_wcast(Wb, ws):
                nc.vector.tensor_copy(Wb[:], ws[:])

            # ones columns of Vn (col 64 of each head block)
            Vn4 = Vn[:].rearrange("p (s h e) -> p s h e", h=HPC, e=VW)
            ones_stage = big.tile([P, ST * HPC], F32)
            nc.vector.memset(ones_stage[:], 1.0)
            nc.vector.tensor_copy(
                Vn4[:, :, :, HD:VW],
                ones_stage[:].rearrange("p (s h e) -> p s h e", h=HPC, e=1),
            )

            # ---- phase 1: X chunks -> transpose -> per-chunk projections ----
            XT3 = XT[:].rearrange("p (k s) -> p k s", s=S)

            def emit_xdma(sp):
                xs = xst.tile([P, 2 * D], F32, tag="xs", name="xs")
                nc.sync.dma_start(
                    _r(xs[:].rearrange("p (t d) -> p t d", d=D)),
                    _r(x[sp * 2 * P:(sp + 1) * 2 * P, :].rearrange(
                        "(t p) d -> p t d", p=P)),
                )
                return xs

            def _emit_tr(sp, xs, tt, g):
                st = sp * 2 + tt
                pt = pp_ss.tile([P, 512], F32, tag="ps", name="pt")
                for j in range(4):
                    kt = g * 4 + j
                    nc.tensor.transpose(
                        _r(pt[:, j * P:(j + 1) * P]),
                        _r(xs[:, tt * D + kt * P: tt * D + (kt + 1) * P]),
                        _r(identr[:]),
                    )
                dst = XT3[:, g * 4:(g + 1) * 4, st * P:(st + 1) * P]
                src = pt[:].rearrange("p (k s) -> p k s", s=P)
                nc.vector.tensor_copy(dst, src)

            def _emit_qk(sp, Wb, Ot, j):
                m0 = sp * 2 * P
                pq = pp_ss.tile([P, 2 * P], F32, tag="ps", name="pq")
                for kt in range(KT):
                    nc.tensor.matmul(
                        pq[:],
                        Wb[:, kt * HG + j * P: kt * HG + (j + 1) * P],
                        XT[:, kt * S + m0: kt * S + m0 + 2 * P],
                        start=(kt == 0),
                        stop=(kt == KT - 1),
                    )
                nc.vector.tensor_copy(Ot[:, j * S + m0: j * S + m0 + 2 * P], pq[:])

            def _emit_v(sp, tt):
                st = sp * 2 + tt
                pv = pp_ss.tile([P, HG], F32, tag="ps", name="pv")
                for kt in range(KT):
                    nc.tensor.matmul(
                        pv[:],
                        XT[:, kt * S + st * P: kt * S + (st + 1) * P],
                        WVb[:, kt * HG:(kt + 1) * HG],
                        start=(kt == 0),
                        stop=(kt == KT - 1),
                    )
                nc.vector.tensor_copy(
                    Vn4[:, st, :, 0:HD], pv[:].rearrange("p (h e) -> p h e", e=HD)
                )

            # DMA order: x0, x1, weights, x2..x7.  Eagerly emit only what the
            # first attention body's first exp needs (transposes + Q j0 of
            # chunks 0-3, K j0 + V of chunk 0); everything else goes into a
            # need-by-ordered unit queue drip-fed into the attention t-loops,
            # so ACT starts exp'ing ~20us in instead of after all of phase 1.
            xs_pend = [emit_xdma(0), emit_xdma(1)]
            ws_q = _emit_wdma(wq)
            ws_k = _emit_wdma(wk)
            ws_v = _emit_wdma(wv)
            _emit_wcast(WQb, ws_q)
            # minimal path to the first exp: transposes + Q j0 of chunks 0-3
            for sp in range(4):
                if sp + 2 < ST // 2:
                    xs_pend.append(emit_xdma(sp + 2))
                for tt in range(2):
                    for g in range(2):
                        _emit_tr(sp, xs_pend[sp], tt, g)
                _emit_qk(sp, WQb, QT, 0)
            # K/V of chunk 0 (needed right after the first exp)
            _emit_wcast(WKb, ws_k)
            _emit_qk(0, WKb, KTt, 0)
            _emit_wcast(WVb, ws_v)
            _emit_v(0, 0)
            _emit_v(0, 1)

            unit_queue = []
            # K j0 + V for chunks 1-3 (needed at t=2,4,6 of the first body)
            for sp in range(1, 4):
                unit_queue.append(lambda sp=sp: _emit_qk(sp, WKb, KTt, 0))
                unit_queue.append(lambda sp=sp: _emit_v(sp, 0))
                unit_queue.append(lambda sp=sp: _emit_v(sp, 1))
            # chunks 4-7 complete (needed from t=8 of the first body)
            for sp in range(4, ST // 2):
                if sp + 2 < ST // 2:
                    unit_queue.append(
                        lambda sp=sp: xs_pend.append(emit_xdma(sp + 2)))
                for tt in range(2):
                    for g in range(2):
                        unit_queue.append(
                            lambda sp=sp, tt=tt, g=g:
                            _emit_tr(sp, xs_pend[sp], tt, g))
                unit_queue.append(lambda sp=sp: _emit_qk(sp, WKb, KTt, 0))
                unit_queue.append(lambda sp=sp: _emit_v(sp, 0))
                unit_queue.append(lambda sp=sp: _emit_v(sp, 1))
                unit_queue.append(lambda sp=sp: _emit_qk(sp, WQb, QT, 0))
            # j1 projections (needed only by the pr=1 bodies)
            for sp in range(ST // 2):
                unit_queue.append(lambda sp=sp: _emit_qk(sp, WKb, KTt, 1))
            for sp in range(ST // 2):
                unit_queue.append(lambda sp=sp: _emit_qk(sp, WQb, QT, 1))

            # ---- phase 2: attention over head pairs ----
            def emit_tail_rest(osbs, pr, mc):
                """Transpose back, normalize, store.  Deferred to just after
                the NEXT block's prologue so the tail's PE transposes (which
                wait on the DVE osb copies) never delay the scores the ACT
                exp chain feeds on.  tr tiles share the "av" tag slots."""
                for half in range(2):
                    h = 2 * pr + half
                    osb = osbs[half]
                    tr = pp_av.tile([P, MC], F32, tag="av", name="tr")
                    ot = outp.tile([P, (MC // P) * HD], F32, tag="ot", name="ot")
                    # all transposes back-to-back first: dep tracking is
                    # coarser than the 65-col packing, so interleaving them
                    # with their DVE readers serializes PE<->DVE per block.
                    for ms in range(MC // P):
                        off = (ms % 4) * VW + (ms // 4) * 512
                        nc.tensor.transpose(
                            tr[:, off:off + VW], osb[0:VW, ms * P:(ms + 1) * P],
                            ident[0:VW, 0:VW],
                        )
                    for ms in range(MC // P):
                        off = (ms % 4) * VW + (ms // 4) * 512
                        pot = tr[:, off:off + VW]
                        rec = recp.tile([P, 1], F32, tag="rec", name="rec")
                        nc.vector.reciprocal(rec[:], pot[:, HD:VW])
                        nc.vector.tensor_scalar_mul(
                            ot[:, ms * HD:(ms + 1) * HD], pot[:, 0:HD], rec[:]
                        )
                    row0 = mc * MC
                    nc.sync.dma_start(
                        out[row0:row0 + MC, h * HD:(h + 1) * HD].rearrange(
                            "(t p) e -> p t e", p=P
                        ),
                        ot[:].rearrange("p (t e) -> p t e", e=HD),
                    )

            def emit_attn_pair(pr, mc, pending_tail):
                j = pr
                qk = []
                for half in range(2):
                    prow = half * 64
                    qk.append((
                        QT[prow:prow + 64, j * S:(j + 1) * S],
                        KTt[prow:prow + 64, j * S:(j + 1) * S],
                    ))

                def mm_scores(half, t):
                    qh, kh = qk[half]
                    ps = pp_ss.tile([P, MC], F32, tag="ps", name="ps")
                    for hf in range(MC // 512):
                        nc.tensor.matmul(
                            ps[:, hf * 512:(hf + 1) * 512],
                            kh[:, t * P:(t + 1) * P],
                            qh[:, mc * MC + hf * 512: mc * MC + (hf + 1) * 512],
                            start=True,
                            stop=True,
                        )
                    return ps

                ps_cur = [mm_scores(0, 0), mm_scores(1, 0)]
                if pending_tail is not None:
                    emit_tail_rest(*pending_tail)
                avs = [
                    pp_av.tile([VW, MC], F32, tag="av", name="av")
                    for _ in range(2)
                ]
                osbs = [None, None]
                for t in range(ST):
                    # drip-feed deferred phase-1 units into the t-loop: PE has
                    # slack vs ACT here, and ACT can start ~50us earlier.
                    for _ in range(3):
                        if unit_queue:
                            unit_queue.pop(0)()
                    for half in range(2):
                        h = 2 * pr + half
                        ps_t = ps_cur[half]
                        if t + 1 < ST:
                            ps_cur[half] = mm_scores(half, t + 1)
                        ex = expp.tile([P, MC], BF16, tag="ex", name="ex")
                        nc.scalar.activation(
                            ex[:], ps_t[:], EXP, scale=1.0 / np.sqrt(HD)
                        )
                        for hf in range(MC // 512):
                            nc.tensor.matmul(
                                avs[half][:, hf * 512:(hf + 1) * 512],
                                Vn[:, t * HPC * VW + h * VW:
                                   t * HPC * VW + (h + 1) * VW],
                                ex[:, hf * 512:(hf + 1) * 512],
                                start=(t == 0),
                                stop=(t == ST - 1),
                            )
                        if t == ST - 1:
                            # evacuate this half's av as soon as it completes
                            osb = osbp.tile([VW, MC], F32, tag="osb", name="osb")
                            nc.vector.tensor_copy(osb[:], avs[half][:])
                            osbs[half] = osb
                return (osbs, pr, mc)

            pending = None
            for pr in range(NPAIR):
                for mc in range(NMC):
                    pending = emit_attn_pair(pr, mc, pending)
            emit_tail_rest(*pending)

    nc.compile()
    return nc


_NC = None


def _get_nc():
    global _NC
    if _NC is None:
        _NC = build_nc()
    return _NC


def _shard_inputs(inputs, Wq, Wk, Wv):
    inputs = np.ascontiguousarray(np.asarray(inputs, dtype=np.float32))
    Wq = np.asarray(Wq, dtype=np.float32)
    Wk = np.asarray(Wk, dtype=np.float32)
    Wv = np.asarray(Wv, dtype=np.float32)
    in_maps = []
    for c in range(NCORES):
        b, g = c // (NCORES // B), c % (NCORES // B)
        sl = slice(g * HG, (g + 1) * HG)
        in_maps.append(
            {
                "x": inputs[b],
                "wq": np.ascontiguousarray(Wq[:, sl]),
                "wk": np.ascontiguousarray(Wk[:, sl]),
                "wv": np.ascontiguousarray(Wv[:, sl]),
            }
        )
    return in_maps


def _gather(results):
    out = np.empty((B, S, H * HD), dtype=np.float32)
    for c in range(NCORES):
        b, g = c // (NCORES // B), c % (NCORES // B)
        out[b, :, g * HG:(g + 1) * HG] = results[c]["out"]
    return out


def kernel(inputs, Wq, Wk, Wv):
    nc = _get_nc()
    in_maps = _shard_inputs(inputs, Wq, Wk, Wv)
    res = run_bass_kernel_spmd(nc, in_maps, core_ids=list(range(NCORES)))
    return _gather(res.results)
